# revision 40
# baseline (speedup 1.0000x reference)
"""Trainium2 Bass kernel for nn_GroupedConvFuseSide4.

out[b,k] = w[k,0]*side5[b,k] + w[k,1]*side4[b,k]
         + w[k,2]*side1[b,0] + w[k,3]*side2[b,0] + w[k,4]*side3[b,0] + bias[k]

Sharding: pure data parallel over batch (B=8) across 8 NeuronCores.

v3 scheme — fp16 wire format + full-128-partition tiles:
  The 262144 pixels of one image are split into CH=64 chunks of FD=4096.
  Row r = 19*g + k (chunk g, class k) gives ROWS=1216 rows of 4096 fp16
  values; tiles of 128 rows (9 full + 64-row tail) stream through SBUF
  with 1 MB DMAs. Per tile:
    - PE: for each of 4 [R,1024] PSUM groups (2 banks), four accumulating
      512-col matmuls: diag(w1) @ x4-tile (per-class scale of side4) and
      lhsT_t @ [ones;singles] (bias + w2*s1 + w3*s2 + w4*s3).
      (weights vary per tile because 128 % 19 != 0 — baked per-tile.)
    - DVE: one scalar_tensor_tensor per group: out = x5*w0 + psum.
  Loads go on the sync+scalar HWDGE queues; stores go on the gpsimd
  queue at [128, 2048] granularity so they interleave with loads and
  never head-of-line block them. All DMA partition counts are 128/64/25
  (counts >64 and !=128 hit degenerate descriptor paths).
  Host converts inputs to fp16 and repacks so every load is a contiguous
  [rows, 8KB] block; output comes back fp16 and is upcast on host.
  Max rel err vs the f32 reference ~8e-4, well under the 2e-2 gate.
"""

import numpy as np

B, K, H, W = 8, 19, 512, 512
FD = 4096                  # pixels per chunk
CH = 64                    # chunks per image (H*W / FD)
ROWS = K * CH              # 1216 packed rows per core
TILES = []                 # (row0, nrows): 9 x 128 + 1 x 64
_r = 0
while _r < ROWS:
    TILES.append((_r, min(128, ROWS - _r)))
    _r += 128
NT = len(TILES)
NGRP = FD // 1024          # 4 psum groups of [R, 1024] (2 banks each)
N_CORES = 8

_cache = {}


def _build_program(w, b):
    import concourse.bacc as bacc
    import concourse.tile as tile
    import concourse.mybir as mybir
    from contextlib import ExitStack

    f16 = mybir.dt.float16
    f32 = mybir.dt.float32
    mult = mybir.AluOpType.mult
    add = mybir.AluOpType.add

    nc = bacc.Bacc(
        "TRN2", target_bir_lowering=False, debug=False,
        enable_asserts=False, num_devices=N_CORES,
    )

    x5_d = nc.dram_tensor("x5", [ROWS, FD], f16, kind="ExternalInput").ap()
    x4_d = nc.dram_tensor("x4", [ROWS, FD], f16, kind="ExternalInput").ap()
    xs_d = nc.dram_tensor("xs", [NT, 25, FD], f16, kind="ExternalInput").ap()
    out_d = nc.dram_tensor("out", [ROWS, FD], f16, kind="ExternalOutput").ap()

    # ---- per-tile baked constants (128 % 19 != 0 so k(p) shifts per tile) ----
    # All f16 consts batched into ONE [128, 256*NT] tensor (tile t: cols
    # [256t,256t+128) = diag(w1), cols [256t+128,256t+256) = singles lhsT)
    # and one [128, NT] f32 tensor for the per-partition w0 scalars, so
    # startup is 2 DMAs instead of 3*NT serialized ones.
    cons16 = np.zeros((128, 256 * NT), dtype=np.float16)
    consw0 = np.zeros((128, NT), dtype=np.float32)
    for t, (r0, R) in enumerate(TILES):
        rr = r0 + np.arange(R)
        kk = rr % K
        gg = rr // K
        g0 = r0 // K
        cons16[np.arange(R), 256 * t + np.arange(R)] = w[kk, 1].astype(np.float16)
        cons16[0, 256 * t + 128:256 * t + 128 + R] = b.astype(np.float16)[kk]
        for s in range(3):
            cons16[1 + 8 * s + (gg - g0),
                   256 * t + 128 + np.arange(R)] = w[kk, 2 + s].astype(np.float16)
        consw0[:R, t] = w[kk, 0]
    cons16_d = nc.inline_tensor(cons16, name="cons16").ap()
    consw0_d = nc.inline_tensor(consw0, name="consw0").ap()

    with tile.TileContext(nc) as tc, ExitStack() as ctx:
        consts = ctx.enter_context(tc.tile_pool(name="consts", bufs=1))
        x5_pool = ctx.enter_context(tc.tile_pool(name="x5", bufs=6))
        x4_pool = ctx.enter_context(tc.tile_pool(name="x4", bufs=6))
        xs_pool = ctx.enter_context(tc.tile_pool(name="xs", bufs=6))
        o_pool = ctx.enter_context(tc.tile_pool(name="o", bufs=5))
        ps_pool = ctx.enter_context(tc.tile_pool(name="ps", bufs=4, space="PSUM"))

        c16 = consts.tile([128, 256 * NT], f16, tag="c16")
        nc.sync.dma_start(out=c16[:], in_=cons16_d)
        cw0 = consts.tile([128, NT], f32, tag="cw0")
        nc.sync.dma_start(out=cw0[:], in_=consw0_d)
        d1_t = [c16[0:R, 256 * t:256 * t + R] for t, (r0, R) in enumerate(TILES)]
        ls_t = [c16[0:25, 256 * t + 128:256 * t + 128 + R]
                for t, (r0, R) in enumerate(TILES)]
        w0_t = [cw0[0:R, t:t + 1] for t, (r0, R) in enumerate(TILES)]

        # process the small 64-row tail tile FIRST: its half-size loads get
        # the compute pipeline started ~2us earlier
        order = [NT - 1] + list(range(NT - 1))
        for t in order:
            r0, R = TILES[t]
            x5t = x5_pool.tile([R, FD], f16, tag="x5")
            nc.sync.dma_start(out=x5t[:], in_=x5_d[r0:r0 + R])
            x4t = x4_pool.tile([R, FD], f16, tag="x4")
            nc.scalar.dma_start(out=x4t[:], in_=x4_d[r0:r0 + R])
            xst = xs_pool.tile([25, FD], f16, tag="xs")
            nc.scalar.dma_start(out=xst[:], in_=xs_d[t])
            ot = o_pool.tile([R, FD], f16, tag="o")

            pss = [ps_pool.tile([R, 1024], f32, tag="ps", name=f"ps{g}")
                   for g in range(NGRP)]
            # group-major matmul order (d1-lo, d1-hi, ls-lo, ls-hi per psum
            # group): each STT becomes ready after 4 matmuls instead of 10,
            # so stores start ~3us earlier per tile.  The PE's two weight
            # slots pipeline the alternating LDWEIGHTS.
            for g in range(NGRP):
                for half in range(2):
                    csl = slice(1024 * g + 512 * half, 1024 * g + 512 * half + 512)
                    nc.tensor.matmul(
                        pss[g][:, 512 * half:512 * half + 512],
                        d1_t[t], x4t[:, csl],
                        start=True, stop=False, skip_group_check=True,
                    )
                for half in range(2):
                    csl = slice(1024 * g + 512 * half, 1024 * g + 512 * half + 512)
                    nc.tensor.matmul(
                        pss[g][:, 512 * half:512 * half + 512],
                        ls_t[t], xst[:, csl],
                        start=False, stop=True, skip_group_check=True,
                    )
                sl = slice(1024 * g, 1024 * (g + 1))
                nc.vector.scalar_tensor_tensor(
                    ot[:, sl], x5t[:, sl], w0_t[t], pss[g][:], mult, add)
                if g % 2 == 1:
                    # store each 2048-col half as soon as its STTs are done
                    osl = slice(2048 * (g // 2), 2048 * (g // 2) + 2048)
                    nc.gpsimd.dma_start(out=out_d[r0:r0 + R, osl],
                                        in_=ot[:, osl])

    nc.compile()
    return nc


def _get_program(w, b):
    key = (w.tobytes(), b.tobytes())
    if key not in _cache:
        _cache[key] = _build_program(w, b)
    return _cache[key]


def _pack_kchw(a16):
    """[K, CH, FD] fp16 -> [ROWS, FD], row = 19*g + k."""
    return np.ascontiguousarray(a16.transpose(1, 0, 2)).reshape(ROWS, FD)


def run(inputs, trace=False, tmpdir=None):
    from concourse.bass_utils import run_bass_kernel_spmd

    w = np.asarray(inputs["weight"], dtype=np.float32)
    b = np.asarray(inputs["bias"], dtype=np.float32)
    nc = _get_program(w, b)

    s1h = np.asarray(inputs["side1"]).astype(np.float16).reshape(B, CH, FD)
    s2h = np.asarray(inputs["side2"]).astype(np.float16).reshape(B, CH, FD)
    s3h = np.asarray(inputs["side3"]).astype(np.float16).reshape(B, CH, FD)
    s4h = np.asarray(inputs["side4"]).astype(np.float16).reshape(B, K, CH, FD)
    s5h = np.asarray(inputs["side5"]).astype(np.float16).reshape(B, K, CH, FD)

    in_maps = []
    for c in range(N_CORES):
        xsp = np.zeros((NT, 25, FD), dtype=np.float16)
        xsp[:, 0] = np.float16(1.0)
        for t, (r0, R) in enumerate(TILES):
            g0 = r0 // K
            g1 = (r0 + R - 1) // K
            n = g1 - g0 + 1
            for s, a in enumerate((s1h[c], s2h[c], s3h[c])):
                xsp[t, 1 + 8 * s:1 + 8 * s + n] = a[g0:g1 + 1]
        in_maps.append({
            "x5": _pack_kchw(s5h[c]),
            "x4": _pack_kchw(s4h[c]),
            "xs": xsp,
        })

    res = run_bass_kernel_spmd(nc, in_maps, list(range(N_CORES)),
                               trace=trace, tmpdir=tmpdir)
    outs = []
    for c in range(N_CORES):
        o = res.results[c]["out"].reshape(CH, K, FD).transpose(1, 0, 2)
        outs.append(o.reshape(1, K, H, W).astype(np.float32))
    return np.concatenate(outs, axis=0), res


def kernel(**inputs):
    out, _ = run(inputs, trace=False)
    return out
